# revision 1
# baseline (speedup 1.0000x reference)
"""Cross-attention Trainium2 kernel (8 NeuronCores, SPMD).

Reference computation (per batch b):
    gate = sigmoid(relu(ctx @ W1 + b1) @ W2 + b2)        # [M, 1]
    ctxg = ctx * gate
    q = x @ Wq; k = ctxg @ Wk; v = ctxg @ Wv             # per head slices of 64
    out = softmax(q k^T / 8) v                           # per head
    y = concat_heads(out) @ Wo + bo                      # [N, 512]

Sharding: 8 cores = 4 batches x 2 query-halves. Each core computes the
FULL output rows for its (batch, 1024-query slice) — no partial sums;
host gather is pure concatenation.

Core-local layout trick: everything is kept transposed (feature dim on
SBUF partitions) so every matmul contraction lands on the partition dim:
    QT[d, i] (d=64/head), KT[d, j], S^T[j, i] = KT_chunk.T @ QT
    E = exp(S^T * scale)  (ScalarE activation doubles as PSUM eviction;
                           no max-subtraction needed: |s| <~ 8 for this data)
    PV: lhsT = [V_h | 1] (ones column) -> out rows 0:64 = V^T E (= O'^T),
        row 64 = colsum(E) = softmax denominator, in the same matmul.
    normalize: O^T = O'^T * (1/denominator) broadcast via a K=1 ones-matmul
        (row 64 becomes exactly 1.0, which then feeds the bias trick below).
    out-proj: y[i, e] = sum_h O^T_h[:, i].T @ Wo_h; head 0 contracts over
        65 rows where row 64 of rhs = bo -> bias added for free.

All matmuls run as float32r (full PE speed at free-dim >= 256, ~fp32
precision).
"""

import os
import sys
from contextlib import ExitStack

import numpy as np

if "/opt/trn_rl_repo" not in sys.path:
    sys.path.insert(0, "/opt/trn_rl_repo")

import concourse.bass as bass
import concourse.mybir as mybir
import concourse.tile as tile
from concourse import bacc
from concourse.bass_utils import run_bass_kernel_spmd
from concourse.masks import make_identity

F32 = mybir.dt.float32
F32R = mybir.dt.float32r
EXPF = mybir.ActivationFunctionType.Exp
RELUF = mybir.ActivationFunctionType.Relu
SIGMF = mybir.ActivationFunctionType.Sigmoid

H = 8          # heads
DH = 64        # dim per head
QD = 512       # query feature dim
CD = 64        # context feature dim
GH = 32        # gate hidden
INNER = H * DH # 512
SCALE = DH ** -0.5


def _r(ap):
    return ap.bitcast(F32R)


def build_core_kernel(nc, NQ=1024, M=2048):
    """Emit the per-core kernel. NQ = queries on this core, M = ctx length."""
    P = 128
    NJC = M // P          # ctx 128-chunks
    NG4 = M // 512        # ctx 512-chunks
    NQC = max(NQ // 512, 1)  # query 512-chunks
    QCW = min(512, NQ)    # query chunk width
    NQ8 = NQ // P         # query 128-chunks
    NKC = QD // P         # 4 qdim 128-chunks

    x_d = nc.dram_tensor("x_in", [NQ, QD], F32, kind="ExternalInput").ap()
    c_d = nc.dram_tensor("ctx_in", [M, CD], F32, kind="ExternalInput").ap()
    wq_d = nc.dram_tensor("wq_in", [QD, INNER], F32, kind="ExternalInput").ap()
    wk_d = nc.dram_tensor("wk_in", [CD, INNER], F32, kind="ExternalInput").ap()
    wv_d = nc.dram_tensor("wv_in", [CD, INNER], F32, kind="ExternalInput").ap()
    wo_d = nc.dram_tensor("wo_in", [INNER, QD], F32, kind="ExternalInput").ap()
    w1_d = nc.dram_tensor("w1_in", [CD, GH], F32, kind="ExternalInput").ap()
    w2_d = nc.dram_tensor("w2_in", [GH, 1], F32, kind="ExternalInput").ap()
    b1_d = nc.dram_tensor("b1_in", [GH, 1], F32, kind="ExternalInput").ap()
    b2_d = nc.dram_tensor("b2_in", [1, 1], F32, kind="ExternalInput").ap()
    bo_d = nc.dram_tensor("bo_in", [1, QD], F32, kind="ExternalInput").ap()
    y_d = nc.dram_tensor("y_out", [NQ, QD], F32, kind="ExternalOutput").ap()

    with TileCtx(nc) as tc, ExitStack() as ctx, \
            nc.allow_low_precision(reason="float32r rounding for PE operands"):
        const = ctx.enter_context(tc.tile_pool(name="const", bufs=1))
        persist = ctx.enter_context(tc.tile_pool(name="persist", bufs=1))
        psum_s = ctx.enter_context(tc.tile_pool(name="psum_s", bufs=2, space="PSUM"))
        psum_pv = ctx.enter_context(tc.tile_pool(name="psum_pv", bufs=2, space="PSUM"))
        psum_pj = ctx.enter_context(tc.tile_pool(name="psum_pj", bufs=2, space="PSUM"))
        early = ExitStack()
        sload = early.enter_context(tc.tile_pool(name="sload", bufs=3))
        gpool = early.enter_context(tc.tile_pool(name="gpool", bufs=2))
        xpool = early.enter_context(tc.tile_pool(name="xpool", bufs=1))

        dma = nc.sync.dma_start

        # ---- constants ----
        ident = const.tile([P, P], F32, tag="ident", name="ident")
        make_identity(nc, ident[:])
        ones_f = const.tile([1, P], F32, tag="ones_f", name="ones_f")
        nc.vector.memset(ones_f[:], 1.0)
        ones = const.tile([1, P], F32R, tag="ones", name="ones")
        nc.vector.tensor_copy(ones[:], ones_f[:])
        onescol_f = const.tile([P, H], F32, tag="onescol_f", name="onescol_f")
        nc.vector.memset(onescol_f[:], 1.0)

        wk_sb = const.tile([CD, INNER], F32R, tag="wk", name="wk")
        dma(wk_sb[:], wk_d[:, :].bitcast(F32R))
        wv_sb = const.tile([CD, INNER], F32R, tag="wv", name="wv")
        dma(wv_sb[:], wv_d[:, :].bitcast(F32R))
        w1_sb = const.tile([CD, GH], F32R, tag="w1", name="w1")
        dma(w1_sb[:], w1_d[:, :].bitcast(F32R))
        w2_sb = const.tile([GH, 1], F32R, tag="w2", name="w2")
        dma(w2_sb[:], w2_d[:, :].bitcast(F32R))
        b1_sb = const.tile([GH, 1], F32, tag="b1", name="b1")
        dma(b1_sb[:], b1_d[:, :])
        b2_sb = const.tile([1, 1], F32, tag="b2", name="b2")
        dma(b2_sb[:], b2_d[:, :])

        # ---- context transpose: ctxT [64, M] ----
        ctxT = persist.tile([CD, M], F32R, tag="ctxT", name="ctxT")
        for g in range(NG4):
            pp = psum_pj.tile([P, 512], F32, tag="pj", name="pj")
            for s in range(4):
                t = g * 4 + s
                cst = sload.tile([P, CD], F32, tag="cld", name="cld")
                dma(cst[:], c_d[t * P:(t + 1) * P, :])
                nc.tensor.transpose(pp[0:CD, s * P:(s + 1) * P], cst[:], ident[:])
            nc.vector.tensor_copy(ctxT[:, g * 512:(g + 1) * 512], pp[0:CD, :])

        # ---- gate + gated context: ctxgT [64, M] ----
        ctxgT = persist.tile([CD, M], F32R, tag="ctxgT", name="ctxgT")
        for g in range(NG4):
            sl = slice(g * 512, (g + 1) * 512)
            pp = psum_pj.tile([P, 512], F32, tag="pj", name="pj")
            nc.tensor.matmul(pp[0:GH, :], _r(w1_sb[:]), _r(ctxT[:, sl]),
                             start=True, stop=True)
            h1 = gpool.tile([GH, 512], F32R, tag="h1", name="h1")
            nc.scalar.activation(h1[:], pp[0:GH, :], RELUF, bias=b1_sb[:])
            pp2 = psum_pj.tile([P, 512], F32, tag="pj", name="pj")
            nc.tensor.matmul(pp2[0:1, :], _r(w2_sb[:]), _r(h1[:]),
                             start=True, stop=True)
            gt = gpool.tile([1, 512], F32R, tag="gt", name="gt")
            nc.scalar.activation(gt[:], pp2[0:1, :], SIGMF, bias=b2_sb[:])
            ppb = psum_pj.tile([P, 512], F32, tag="pj", name="pj")
            nc.tensor.matmul(ppb[0:CD, :], _r(ones[:, 0:CD]), _r(gt[:]),
                             start=True, stop=True)
            nc.vector.tensor_mul(ctxgT[:, sl], ctxT[:, sl], ppb[0:CD, :])

        # ---- K^T, head-pair stacked: KT[pr] [128, M] (rows 0:64 = head 2pr) ----
        KT = [persist.tile([P, M], F32R, tag=f"kt{pr}", name=f"kt{pr}") for pr in range(H // 2)]
        for pr in range(H // 2):
            for g in range(NG4):
                sl = slice(g * 512, (g + 1) * 512)
                pp = psum_pj.tile([P, 512], F32, tag="pj", name="pj")
                nc.tensor.matmul(pp[:], _r(wk_sb[:, pr * P:(pr + 1) * P]),
                                 _r(ctxgT[:, sl]), start=True, stop=True)
                nc.vector.tensor_copy(KT[pr][:, sl], pp[:])

        # ---- V natural, interleaved [V_h | 1] blocks of 65: Vt[t] [128, 520] ----
        Vt = [persist.tile([P, H * (DH + 1)], F32R, tag=f"v{t}", name=f"v{t}") for t in range(NJC)]
        for t in range(NJC):
            vv = Vt[t][:].rearrange("p (h c) -> p h c", c=DH + 1)
            nc.vector.tensor_copy(
                vv[:, :, DH:DH + 1],
                onescol_f[:].rearrange("p (h o) -> p h o", o=1))
            pp = psum_pj.tile([P, 512], F32, tag="pj", name="pj")
            nc.tensor.matmul(pp[:], _r(ctxgT[:, t * P:(t + 1) * P]), _r(wv_sb[:]),
                             start=True, stop=True)
            nc.vector.tensor_copy(
                vv[:, :, 0:DH],
                pp[:].rearrange("p (h c) -> p h c", c=DH))

        # ---- x transpose + Q^T (head-pair stacked): QT[pr] [128, NQ] ----
        # x/wq ride the Activation-engine HWDGE queue so they overlap the
        # ctx-chain DMAs on the SP queue.
        dma2 = nc.scalar.dma_start
        wq_sb = [const.tile([P, INNER], F32R, tag=f"wq{k}", name=f"wq{k}") for k in range(NKC)]
        for k in range(NKC):
            dma2(wq_sb[k][:], wq_d[k * P:(k + 1) * P, :].bitcast(F32R))
        xT = [xpool.tile([P, NQ], F32R, tag=f"xT{k}", name=f"xT{k}") for k in range(NKC)]
        for q8 in range(NQ8):
            xst = sload.tile([P, QD], F32, tag="xld", name="xld")
            dma2(xst[:], x_d[q8 * P:(q8 + 1) * P, :])
            pp = psum_pj.tile([P, 512], F32, tag="pj", name="pj")
            for k in range(NKC):
                nc.tensor.transpose(pp[:, k * P:(k + 1) * P],
                                    xst[:, k * P:(k + 1) * P], ident[:])
            for k in range(NKC):
                nc.vector.tensor_copy(xT[k][:, q8 * P:(q8 + 1) * P],
                                      pp[:, k * P:(k + 1) * P])
        QT = [persist.tile([P, NQ], F32R, tag=f"qt{pr}", name=f"qt{pr}") for pr in range(H // 2)]
        for pr in range(H // 2):
            for qc in range(NQC):
                sl = slice(qc * QCW, (qc + 1) * QCW)
                pp = psum_pj.tile([P, 512], F32, tag="pj", name="pj")
                for k in range(NKC):
                    nc.tensor.matmul(pp[:, 0:QCW],
                                     _r(wq_sb[k][:, pr * P:(pr + 1) * P]),
                                     _r(xT[k][:, sl]),
                                     start=(k == 0), stop=(k == NKC - 1))
                nc.vector.tensor_copy(QT[pr][:, sl], pp[:, 0:QCW])

        early.close()
        epool = ctx.enter_context(tc.tile_pool(name="epool", bufs=3))
        rpool = ctx.enter_context(tc.tile_pool(name="rpool", bufs=2))
        wopool = ctx.enter_context(tc.tile_pool(name="wopool", bufs=1))
        # Wo per head; head 0 gets a 65th row holding bo (bias via ones-row)
        wo_sb = []
        for h in range(H):
            t = wopool.tile([DH + 1 if h == 0 else DH, QD], F32R, tag=f"wo{h}", name=f"wo{h}")
            dma(t[0:DH, :], wo_d[h * DH:(h + 1) * DH, :].bitcast(F32R))
            if h == 0:
                dma(t[DH:DH + 1, :], bo_d[:, :].bitcast(F32R))
            wo_sb.append(t)

        # ---- attention per head ----
        OT = [persist.tile([DH + 1, NQ], F32R, tag=f"ot{h}", name=f"ot{h}") for h in range(H)]
        for h in range(H):
            pr, lo = h // 2, (h % 2) * DH
            pv = [psum_pv.tile([DH + 1, 512], F32, tag="pv", name="pv") for _ in range(NQC)]
            for jc in range(NJC):
                st = psum_s.tile([P, NQC * 512], F32, tag="s", name="st")
                for qc in range(NQC):
                    nc.tensor.matmul(
                        st[:, qc * 512:qc * 512 + QCW],
                        _r(KT[pr][lo:lo + DH, jc * P:(jc + 1) * P]),
                        _r(QT[pr][lo:lo + DH, qc * QCW:(qc + 1) * QCW]),
                        start=True, stop=True)
                et = epool.tile([P, NQC * 512], F32R, tag="e", name="et")
                nc.scalar.activation(et[:], st[:], EXPF, scale=SCALE)
                for qc in range(NQC):
                    nc.tensor.matmul(
                        pv[qc][:, 0:QCW],
                        _r(Vt[jc][:, h * (DH + 1):(h + 1) * (DH + 1)]),
                        _r(et[:, qc * 512:qc * 512 + QCW]),
                        start=(jc == 0), stop=(jc == NJC - 1))
            # fast eviction only — frees the pv banks so the next head's
            # accumulation starts immediately; normalization is deferred.
            for qc in range(NQC):
                sl = slice(qc * QCW, (qc + 1) * QCW)
                nc.vector.tensor_copy(OT[h][:, sl], pv[qc][:, 0:QCW])

        # ---- deferred normalize + output projection, interleaved by qc ----
        # O^T rows 0:64 /= denom (row 64 -> exactly 1.0, feeding the bias
        # trick); then project the q-chunks of this qc while the next qc
        # normalizes.
        for qc in range(NQC):
            sl = slice(qc * QCW, (qc + 1) * QCW)
            for h in range(H):
                rec = rpool.tile([1, 512], F32R, tag="rec", name="rec")
                nc.vector.reciprocal(rec[:, 0:QCW],
                                     OT[h][DH:DH + 1, sl].bitcast(F32))
                rb = psum_pj.tile([DH + 1, 512], F32, tag="pj", name="rb")
                nc.tensor.matmul(rb[:, 0:QCW], _r(ones[:, 0:DH + 1]),
                                 _r(rec[:, 0:QCW]), start=True, stop=True)
                rbs = rpool.tile([DH + 1, 512], F32, tag="rbs", name="rbs")
                nc.vector.tensor_copy(rbs[:, 0:QCW], rb[:, 0:QCW])
                nc.vector.tensor_mul(OT[h][:, sl], OT[h][:, sl].bitcast(F32),
                                     rbs[:, 0:QCW])
            for q8 in range(qc * QCW // P, (qc + 1) * QCW // P):
                po = psum_pj.tile([P, 512], F32, tag="pj", name="pj")
                for h in range(H):
                    kk = DH + 1 if h == 0 else DH
                    nc.tensor.matmul(po[:],
                                     _r(OT[h][0:kk, q8 * P:(q8 + 1) * P]),
                                     _r(wo_sb[h][0:kk, :]),
                                     start=(h == 0), stop=(h == H - 1))
                ost = rpool.tile([P, 512], F32, tag="ost", name="ost")
                nc.vector.tensor_copy(ost[:], po[:])
                dma(y_d[q8 * P:(q8 + 1) * P, :], ost[:])

    return nc


def TileCtx(nc):
    return tile.TileContext(nc)


_NC_CACHE = {}


def _get_compiled(NQ=1024, M=2048):
    key = (NQ, M)
    if key not in _NC_CACHE:
        nc = bacc.Bacc("TRN2", target_bir_lowering=False, debug=False)
        build_core_kernel(nc, NQ=NQ, M=M)
        nc.compile()
        _NC_CACHE[key] = nc
    return _NC_CACHE[key]


def _make_in_maps(inputs):
    x = np.ascontiguousarray(np.asarray(inputs["x"], dtype=np.float32))
    context = np.ascontiguousarray(np.asarray(inputs["context"], dtype=np.float32))
    B, N, _ = x.shape
    NQ = N // 2
    common = {
        "wq_in": np.asarray(inputs["Wq"], np.float32),
        "wk_in": np.asarray(inputs["Wk"], np.float32),
        "wv_in": np.asarray(inputs["Wv"], np.float32),
        "wo_in": np.asarray(inputs["Wo"], np.float32),
        "w1_in": np.asarray(inputs["W1"], np.float32),
        "w2_in": np.asarray(inputs["W2"], np.float32).reshape(GH, 1),
        "b1_in": np.asarray(inputs["b1"], np.float32).reshape(GH, 1),
        "b2_in": np.asarray(inputs["b2"], np.float32).reshape(1, 1),
        "bo_in": np.asarray(inputs["bo"], np.float32).reshape(1, QD),
    }
    in_maps = []
    for c in range(8):
        b, qh = c // 2, c % 2
        m = dict(common)
        m["x_in"] = np.ascontiguousarray(x[b, qh * NQ:(qh + 1) * NQ, :])
        m["ctx_in"] = np.ascontiguousarray(context[b])
        in_maps.append(m)
    return in_maps


def kernel(x, context, Wq, Wk, Wv, W1, b1, W2, b2, Wo, bo):
    x = np.ascontiguousarray(np.asarray(x, dtype=np.float32))
    context = np.ascontiguousarray(np.asarray(context, dtype=np.float32))
    B, N, _ = x.shape
    NQ = N // 2
    M = context.shape[1]
    nc = _get_compiled(NQ=NQ, M=M)
    in_maps = _make_in_maps(dict(
        x=x, context=context, Wq=Wq, Wk=Wk, Wv=Wv, W1=W1, b1=b1, W2=W2,
        b2=b2, Wo=Wo, bo=bo))

    res = run_bass_kernel_spmd(nc, in_maps, list(range(8))).results
    out = np.empty((B, N, QD), dtype=np.float32)
    for c in range(8):
        b, qh = c // 2, c % 2
        out[b, qh * NQ:(qh + 1) * NQ, :] = res[c]["y_out"]
    return out



# revision 7
# speedup vs baseline: 1.6768x; 1.6768x over previous
"""Cross-attention Trainium2 kernel (8 NeuronCores, SPMD).

Reference computation (per batch b):
    gate = sigmoid(relu(ctx @ W1 + b1) @ W2 + b2)        # [M, 1]
    ctxg = ctx * gate
    q = x @ Wq; k = ctxg @ Wk; v = ctxg @ Wv             # per head slices of 64
    out = softmax(q k^T / 8) v                           # per head
    y = concat_heads(out) @ Wo + bo                      # [N, 512]

Sharding: 8 cores = 4 batches x 2 query-halves. Each core computes the
FULL output rows for its (batch, 1024-query slice); host gather is pure
concatenation.

Core-local design (v2 — PE-density-first):
  * Everything transposed: QT[d,i], KT[d,j] head-pair stacked on 128
    partitions; S^T[j,i] = KT_chunk.T @ QT; E = exp(S^T*scale) (ScalarE,
    PSUM->SBUF, bf16); PV: lhsT=[V_h|1] -> O'^T rows + denominator row in
    one accumulation chain.
  * All matmul operands bf16 (FWL halves LDWEIGHTS; rel err ~5e-3 vs 2e-2
    budget), PSUM accumulation fp32.
  * Flat 128-unit software pipeline over (head, jc): scores run 2 units
    ahead of PV so the scores->exp->PV cross-engine chain (2 sem hops +
    ~1us exp) never stalls the PE queue.
  * Q/K projections for the next head-pair, V projection, out-projection
    partial sums for the previous pair, and the softmax normalization are
    interleaved into the attention stream as PE "filler" tasks: the PE
    never idles, which keeps the HAM clock gate at K=8/8 (2.4 GHz). The
    v1 kernel ran 72% of its span at K=4/8 (1.2 GHz) because the
    phase-separated layout left the PE 15-50% idle in every phase.
  * Normalization: denominator row broadcast via ones-matmul (PE), then
    DVE reciprocal_approx_fast on all 65 partitions (replaces v1's
    serial [1,512] nc.vector.reciprocal: 53us -> ~2us), then one DVE
    multiply fused over O'^T (row 64 -> exactly 1.0, feeding the bias
    trick: head 0's Wo gets a 65th row holding bo).
  * Out-projection accumulated per head-pair into PSUM, folded into an
    SBUF fp32 running sum (DVE), so the epilogue is just the last pair.
"""

import sys

import numpy as np

if "/opt/trn_rl_repo" not in sys.path:
    sys.path.insert(0, "/opt/trn_rl_repo")

import concourse.bass as bass
import concourse.mybir as mybir
import concourse.tile as tile
from concourse import bacc
from concourse.bass_utils import run_bass_kernel_spmd
from concourse.masks import make_identity
from contextlib import ExitStack

F32 = mybir.dt.float32
F32R = mybir.dt.float32r
BF16 = mybir.dt.bfloat16
EXPF = mybir.ActivationFunctionType.Exp
RELUF = mybir.ActivationFunctionType.Relu
SIGMF = mybir.ActivationFunctionType.Sigmoid

H = 8          # heads
DH = 64        # dim per head
QD = 512       # query feature dim
CD = 64        # context feature dim
GH = 32        # gate hidden
INNER = H * DH # 512
SCALE = DH ** -0.5
P = 128


def _r(ap):
    return ap.bitcast(F32R)


def build_core_kernel(nc, NQ=1024, M=2048):
    NJC = M // P          # 16 ctx 128-chunks
    NG4 = M // 512        # 4 ctx 512-chunks
    NQ8 = NQ // P         # 8 query 128-chunks
    NKC = QD // P         # 4 qdim 128-chunks
    NPR = H // 2          # 4 head pairs
    UNITS = [(h, jc) for h in range(H) for jc in range(NJC)]
    NU = len(UNITS)       # 128

    x_d = nc.dram_tensor("x_in", [NQ, QD], F32, kind="ExternalInput").ap()
    c_d = nc.dram_tensor("ctx_in", [M, CD], F32, kind="ExternalInput").ap()
    wq_d = nc.dram_tensor("wq_in", [QD, INNER], F32, kind="ExternalInput").ap()
    wk_d = nc.dram_tensor("wk_in", [CD, INNER], F32, kind="ExternalInput").ap()
    wv_d = nc.dram_tensor("wv_in", [CD, INNER], F32, kind="ExternalInput").ap()
    wo_d = nc.dram_tensor("wo_in", [INNER, QD], F32, kind="ExternalInput").ap()
    w1_d = nc.dram_tensor("w1_in", [CD, GH], F32, kind="ExternalInput").ap()
    w2_d = nc.dram_tensor("w2_in", [GH, 1], F32, kind="ExternalInput").ap()
    b1_d = nc.dram_tensor("b1_in", [GH, 1], F32, kind="ExternalInput").ap()
    b2_d = nc.dram_tensor("b2_in", [1, 1], F32, kind="ExternalInput").ap()
    bo_d = nc.dram_tensor("bo_in", [1, QD], F32, kind="ExternalInput").ap()
    y_d = nc.dram_tensor("y_out", [NQ, QD], F32, kind="ExternalOutput").ap()

    with tile.TileContext(nc) as tc, ExitStack() as ctx, \
            nc.allow_low_precision(reason="bf16 operands / f32r transposes"):
        const = ctx.enter_context(tc.tile_pool(name="const", bufs=1))
        persist = ctx.enter_context(tc.tile_pool(name="persist", bufs=1))
        wstage = ctx.enter_context(tc.tile_pool(name="wstage", bufs=2))
        sloadx = ctx.enter_context(tc.tile_pool(name="sloadx", bufs=4))
        sloadc = ctx.enter_context(tc.tile_pool(name="sloadc", bufs=4))
        gpool = ctx.enter_context(tc.tile_pool(name="gpool", bufs=2))
        epool = ctx.enter_context(tc.tile_pool(name="epool", bufs=3))
        rpool = ctx.enter_context(tc.tile_pool(name="rpool", bufs=4))
        opool = ctx.enter_context(tc.tile_pool(name="opool", bufs=2))
        psum_st = ctx.enter_context(tc.tile_pool(name="psum_st", bufs=2, space="PSUM"))
        psum_pv = ctx.enter_context(tc.tile_pool(name="psum_pv", bufs=2, space="PSUM"))
        psum_pj = ctx.enter_context(tc.tile_pool(name="psum_pj", bufs=2, space="PSUM"))

        dma = nc.sync.dma_start       # SP HWDGE queue: ctx, wk/wv/gate, wo, y out
        dma2 = nc.scalar.dma_start    # ACT HWDGE queue: x, then wq
        dma3 = dma2

        # ---- constants ----
        ident = const.tile([P, P], F32, tag="ident", name="ident")
        make_identity(nc, ident[:])
        ones_bf = const.tile([P, P], BF16, tag="ones_bf", name="ones_bf")
        nc.gpsimd.memset(ones_bf[:], 1.0)

        # ---- DMA: ctx (first use), gate/kv weights, wo; x + wq on other queues
        cld = []
        for g in range(NG4):
            t = sloadc.tile([P, 4, CD], F32, tag="cld", name="cld")
            dma(t[:], c_d[g * 512:(g + 1) * 512, :].rearrange("(s p) c -> p s c", p=P))
            cld.append(t)
        wk_st = wstage.tile([CD, INNER], F32, tag="wkv", name="wk_st")
        dma(wk_st[:], wk_d[:, :])
        wv_st = wstage.tile([CD, INNER], F32, tag="wkv", name="wv_st")
        dma(wv_st[:], wv_d[:, :])
        w1_st = wstage.tile([CD, GH], F32, tag="wg", name="w1_st")
        dma(w1_st[:], w1_d[:, :])
        w2_st = wstage.tile([GH, 1], F32, tag="wg", name="w2_st")
        dma(w2_st[:], w2_d[:, :])
        b1_sb = const.tile([GH, 1], F32, tag="b1", name="b1")
        dma(b1_sb[:], b1_d[:, :])
        b2_sb = const.tile([1, 1], F32, tag="b2", name="b2")
        dma(b2_sb[:], b2_d[:, :])

        xst = []
        for q8 in range(NQ8):
            t = sloadx.tile([P, QD], F32, tag="xld", name="xld")
            dma2(t[:], x_d[q8 * P:(q8 + 1) * P, :])
            xst.append(t)

        # bf16 weight copies
        wk_sb = const.tile([CD, INNER], BF16, tag="wk", name="wk")
        nc.any.tensor_copy(wk_sb[:], wk_st[:])
        wv_sb = const.tile([CD, INNER], BF16, tag="wv", name="wv")
        nc.any.tensor_copy(wv_sb[:], wv_st[:])
        w1_sb = const.tile([CD, GH], BF16, tag="w1", name="w1")
        nc.any.tensor_copy(w1_sb[:], w1_st[:])
        w2_sb = const.tile([GH, 1], BF16, tag="w2", name="w2")
        nc.any.tensor_copy(w2_sb[:], w2_st[:])

        wq_sb = []
        for k in range(NKC):
            st = wstage.tile([P, INNER], F32, tag="wqst", name="wq_st")
            dma3(st[:], wq_d[k * P:(k + 1) * P, :])
            t = const.tile([P, INNER], BF16, tag=f"wq{k}", name=f"wq{k}")
            nc.any.tensor_copy(t[:], st[:])
            wq_sb.append(t)

        wo_sb = []
        for h in range(H):
            st = wstage.tile([DH, QD], F32, tag="wost", name="wo_st")
            dma(st[:], wo_d[h * DH:(h + 1) * DH, :])
            t = const.tile([DH + 1 if h == 0 else DH, QD], BF16,
                           tag=f"wo{h}", name=f"wo{h}")
            nc.any.tensor_copy(t[0:DH, :], st[:])
            wo_sb.append(t)
        bo_st = wstage.tile([1, QD], F32, tag="bost", name="bo_st")
        dma(bo_st[:], bo_d[:, :])
        nc.any.tensor_copy(wo_sb[0][DH:DH + 1, :], bo_st[:])

        # ---- persistent SBUF tensors (bf16) ----
        ctxT = persist.tile([CD, M], BF16, tag="ctxT", name="ctxT")
        ctxgT = persist.tile([CD, M], BF16, tag="ctxgT", name="ctxgT")
        xT = [persist.tile([P, NQ], BF16, tag=f"xT{k}", name=f"xT{k}") for k in range(NKC)]
        KT = [persist.tile([P, M], BF16, tag=f"kt{pr}", name=f"kt{pr}") for pr in range(NPR)]
        QT = [persist.tile([P, NQ], BF16, tag=f"qt{pr}", name=f"qt{pr}") for pr in range(NPR)]
        Vt = [persist.tile([P, H * (DH + 1)], BF16, tag=f"v{t}", name=f"v{t}") for t in range(NJC)]
        OT = [persist.tile([DH + 1, NQ], BF16, tag=f"ot{h}", name=f"ot{h}") for h in range(H)]
        ypart = [persist.tile([P, QD], F32, tag=f"yp{q8}", name=f"yp{q8}") for q8 in range(NQ8)]

        # Vt ones-columns (denominator trick)
        for t in range(NJC):
            vv = Vt[t][:].rearrange("p (h c) -> p h c", c=DH + 1)
            nc.gpsimd.memset(vv[:, :, DH:DH + 1], 1.0)

        # ---- prologue: ctx transpose ----
        for g in range(NG4):
            pp = psum_pj.tile([P, 512], F32, tag="pj", name="pj")
            for s in range(4):
                nc.tensor.transpose(pp[0:CD, s * P:(s + 1) * P],
                                    cld[g][:, s, :], ident[:])
            nc.any.tensor_copy(ctxT[:, g * 512:(g + 1) * 512], pp[0:CD, :])

        # ---- gate + gated context ----
        for g in range(NG4):
            sl = slice(g * 512, (g + 1) * 512)
            pp = psum_pj.tile([P, 512], F32, tag="pj", name="pj")
            nc.tensor.matmul(pp[0:GH, :], w1_sb[:], ctxT[:, sl], start=True, stop=True)
            h1 = gpool.tile([GH, 512], BF16, tag="h1", name="h1")
            nc.scalar.activation(h1[:], pp[0:GH, :], RELUF, bias=b1_sb[:])
            pp2 = psum_pj.tile([P, 512], F32, tag="pj", name="pj")
            nc.tensor.matmul(pp2[0:1, :], w2_sb[:], h1[:], start=True, stop=True)
            gt = gpool.tile([1, 512], BF16, tag="gt", name="gt")
            nc.scalar.activation(gt[:], pp2[0:1, :], SIGMF, bias=b2_sb[:])
            ppb = psum_pj.tile([P, 512], F32, tag="pj", name="pj")
            nc.tensor.matmul(ppb[0:CD, :], ones_bf[0:1, 0:CD], gt[:], start=True, stop=True)
            nc.vector.tensor_mul(ctxgT[:, sl], ctxT[:, sl], ppb[0:CD, :])

        # ---- prologue: x transpose ----
        for q8 in range(NQ8):
            pp = psum_pj.tile([P, 512], F32, tag="pj", name="pj")
            for k in range(NKC):
                nc.tensor.transpose(pp[:, k * P:(k + 1) * P],
                                    xst[q8][:, k * P:(k + 1) * P], ident[:])
            for k in range(NKC):
                nc.any.tensor_copy(xT[k][:, q8 * P:(q8 + 1) * P],
                                   pp[:, k * P:(k + 1) * P])

        # ---- projection task emitters (used in prologue + as PE filler) ----
        def emit_qt(pr, qc):
            sl = slice(qc * 512, (qc + 1) * 512)
            pp = psum_pj.tile([P, 512], F32, tag="pj", name="pj")
            for k in range(NKC):
                nc.tensor.matmul(pp[:], wq_sb[k][:, pr * P:(pr + 1) * P],
                                 xT[k][:, sl], start=(k == 0), stop=(k == NKC - 1))
            nc.vector.tensor_copy(QT[pr][:, sl], pp[:])

        def emit_kt(pr, g):
            sl = slice(g * 512, (g + 1) * 512)
            pp = psum_pj.tile([P, 512], F32, tag="pj", name="pj")
            nc.tensor.matmul(pp[:], wk_sb[:, pr * P:(pr + 1) * P], ctxgT[:, sl],
                             start=True, stop=True)
            nc.vector.tensor_copy(KT[pr][:, sl], pp[:])

        def emit_vt(t):
            vv = Vt[t][:].rearrange("p (h c) -> p h c", c=DH + 1)
            pp = psum_pj.tile([P, 512], F32, tag="pj", name="pj")
            nc.tensor.matmul(pp[:], ctxgT[:, t * P:(t + 1) * P], wv_sb[:],
                             start=True, stop=True)
            nc.vector.tensor_copy(vv[:, :, 0:DH], pp[:].rearrange("p (h c) -> p h c", c=DH))

        def emit_outproj(pr, q8):
            # partial y for head pair pr, query chunk q8 -> fold into ypart
            pp = psum_pj.tile([P, 512], F32, tag="pj", name="pj")
            for i, h in enumerate((2 * pr, 2 * pr + 1)):
                kk = DH + 1 if h == 0 else DH
                nc.tensor.matmul(pp[:], OT[h][0:kk, q8 * P:(q8 + 1) * P],
                                 wo_sb[h][0:kk, :], start=(i == 0), stop=(i == 1))
            if pr == 0:
                nc.vector.tensor_copy(ypart[q8][:], pp[:])
            else:
                nc.vector.tensor_add(ypart[q8][:], ypart[q8][:], pp[:])

        # ---- prologue: first pair projections + first V chunks ----
        VT_PRE = 3
        for t in range(VT_PRE):
            emit_vt(t)
        for g in range(NG4):
            emit_kt(0, g)
        for qc in range(2):
            emit_qt(0, qc)

        # ---- filler task queues per pair-block ----
        blocks = [[] for _ in range(NPR)]
        for t in range(VT_PRE, NJC):
            blocks[0].append(lambda t=t: emit_vt(t))
        for pr in range(1, NPR):
            for qc in range(2):
                blocks[pr - 1].append(lambda pr=pr, qc=qc: emit_qt(pr, qc))
            for g in range(NG4):
                blocks[pr - 1].append(lambda pr=pr, g=g: emit_kt(pr, g))
        for pr in range(NPR - 1):
            for q8 in range(NQ8):
                blocks[pr + 1].append(lambda pr=pr, q8=q8: emit_outproj(pr, q8))

        # ---- attention: flat pipeline, PV lags scores by 2 units ----
        st_tiles = {}
        e_tiles = {}
        pv_tiles = {}

        def emit_scores(i):
            h, jc = UNITS[i]
            pr, lo = h // 2, (h % 2) * DH
            st = psum_st.tile([P, 1024], F32, tag="st", name="st")
            for qc in range(2):
                nc.tensor.matmul(
                    st[:, qc * 512:(qc + 1) * 512],
                    KT[pr][lo:lo + DH, jc * P:(jc + 1) * P],
                    QT[pr][lo:lo + DH, qc * 512:(qc + 1) * 512],
                    start=True, stop=True)
            st_tiles[i] = st
            et = epool.tile([P, 1024], BF16, tag="e", name="et")
            nc.scalar.activation(et[:], st[:], EXPF, scale=SCALE)
            e_tiles[i] = et

        def emit_pv(i):
            h, jc = UNITS[i]
            if jc == 0:
                pv_tiles[h] = [psum_pv.tile([DH + 1, 512], F32, tag="pv", name="pv")
                               for _ in range(2)]
            et = e_tiles.pop(i)
            st_tiles.pop(i)
            for qc in range(2):
                nc.tensor.matmul(
                    pv_tiles[h][qc][:],
                    Vt[jc][:, h * (DH + 1):(h + 1) * (DH + 1)],
                    et[:, qc * 512:(qc + 1) * 512],
                    start=(jc == 0), stop=(jc == NJC - 1))

        def emit_head_tail(h):
            # evict raw O'^T (+denominator row), broadcast denom, approx-recip,
            # normalize in place (row 64 -> 1.0 for the bias trick)
            pv = pv_tiles.pop(h)
            for qc in range(2):
                sl = slice(qc * 512, (qc + 1) * 512)
                nc.vector.tensor_copy(OT[h][:, sl], pv[qc][:])
            rbs = []
            for qc in range(2):
                sl = slice(qc * 512, (qc + 1) * 512)
                rb = psum_pj.tile([P, 512], F32, tag="pj", name="pj")
                nc.tensor.matmul(rb[0:DH + 1, :], ones_bf[64:65, 0:DH + 1],
                                 OT[h][DH:DH + 1, sl], start=True, stop=True)
                rr = rpool.tile([DH + 1, 512], F32, tag="rbs", name="rbs")
                nc.vector.reciprocal_approx_fast(rr[:], rb[0:DH + 1, :])
                rbs.append(rr)
            for qc in range(2):
                sl = slice(qc * 512, (qc + 1) * 512)
                nc.vector.tensor_mul(OT[h][:, sl], OT[h][:, sl], rbs[qc][:])

        emit_scores(0)
        emit_scores(1)
        for i in range(2, NU):
            emit_scores(i)
            emit_pv(i - 2)
            h, jc = UNITS[i - 2]
            if jc == NJC - 1:
                emit_head_tail(h)
            # block 3's fillers (pair-2 out-proj) need head 5's tail, emitted
            # at i=97 — delay its pops so the intended PE order holds.
            blk = blocks[i // 32]
            if blk and (i // 32 < 3 or i % 32 >= 6):
                blk.pop(0)()
        emit_pv(NU - 2)
        emit_pv(NU - 1)
        emit_head_tail(H - 1)
        for blk in blocks:
            while blk:
                blk.pop(0)()

        # ---- epilogue: last pair out-projection + final sum + store ----
        for q8 in range(NQ8):
            pp = psum_pj.tile([P, 512], F32, tag="pj", name="pj")
            for i, h in enumerate((H - 2, H - 1)):
                nc.tensor.matmul(pp[:], OT[h][0:DH, q8 * P:(q8 + 1) * P],
                                 wo_sb[h][0:DH, :], start=(i == 0), stop=(i == 1))
            ost = opool.tile([P, QD], F32, tag="ost", name="ost")
            nc.vector.tensor_add(ost[:], ypart[q8][:], pp[:])
            dma(y_d[q8 * P:(q8 + 1) * P, :], ost[:])

    return nc


_NC_CACHE = {}


def _get_compiled(NQ=1024, M=2048):
    key = (NQ, M)
    if key not in _NC_CACHE:
        nc = bacc.Bacc("TRN2", target_bir_lowering=False, debug=False)
        build_core_kernel(nc, NQ=NQ, M=M)
        nc.compile()
        _NC_CACHE[key] = nc
    return _NC_CACHE[key]


def _make_in_maps(inputs):
    x = np.ascontiguousarray(np.asarray(inputs["x"], dtype=np.float32))
    context = np.ascontiguousarray(np.asarray(inputs["context"], dtype=np.float32))
    B, N, _ = x.shape
    NQ = N // 2
    common = {
        "wq_in": np.asarray(inputs["Wq"], np.float32),
        "wk_in": np.asarray(inputs["Wk"], np.float32),
        "wv_in": np.asarray(inputs["Wv"], np.float32),
        "wo_in": np.asarray(inputs["Wo"], np.float32),
        "w1_in": np.asarray(inputs["W1"], np.float32),
        "w2_in": np.asarray(inputs["W2"], np.float32).reshape(GH, 1),
        "b1_in": np.asarray(inputs["b1"], np.float32).reshape(GH, 1),
        "b2_in": np.asarray(inputs["b2"], np.float32).reshape(1, 1),
        "bo_in": np.asarray(inputs["bo"], np.float32).reshape(1, QD),
    }
    in_maps = []
    for c in range(8):
        b, qh = c // 2, c % 2
        m = dict(common)
        m["x_in"] = np.ascontiguousarray(x[b, qh * NQ:(qh + 1) * NQ, :])
        m["ctx_in"] = np.ascontiguousarray(context[b])
        in_maps.append(m)
    return in_maps


def kernel(x, context, Wq, Wk, Wv, W1, b1, W2, b2, Wo, bo):
    x = np.ascontiguousarray(np.asarray(x, dtype=np.float32))
    context = np.ascontiguousarray(np.asarray(context, dtype=np.float32))
    B, N, _ = x.shape
    NQ = N // 2
    M = context.shape[1]
    nc = _get_compiled(NQ=NQ, M=M)
    in_maps = _make_in_maps(dict(
        x=x, context=context, Wq=Wq, Wk=Wk, Wv=Wv, W1=W1, b1=b1, W2=W2,
        b2=b2, Wo=Wo, bo=bo))

    res = run_bass_kernel_spmd(nc, in_maps, list(range(8))).results
    out = np.empty((B, N, QD), dtype=np.float32)
    for c in range(8):
        b, qh = c // 2, c % 2
        out[b, qh * NQ:(qh + 1) * NQ, :] = res[c]["y_out"]
    return out
